# revision 11
# baseline (speedup 1.0000x reference)
"""Grouped-Query Attention forward pass on 8 Trainium2 NeuronCores.

Sharding: 2-way data parallel over batch x 4-way tensor parallel over KV
head groups. Core c = 4*b + g handles batch b and KV group g (4 query
heads + 1 KV head). Each core computes a partial o-projection output
(its head group's contribution, full [S, D]); the host sums the 4
partials per batch.

v3 design (vs 424us v1 baseline):
  - bf16 matmuls for projections / scores / o-proj (same 1 cycle/row PE
    rate as float32r, halves DMA+SBUF).  exp and AV stay float32r: the
    ACT engine writes bf16 at half rate, so bf16 exp tiles are a loss.
  - All DRAM operands host-pre-swizzled to [128-partition, long-line]
    layouts (4-16KB contiguous per partition per DMA descriptor); v1
    used 256-512B lines and spent 46us of prologue on ~52k descriptors.
    First x tile + chunked Wq are issued before everything else so PE
    starts at ~6us.
  - Softmax denominator stays on PE (per-key-tile ones-matmul into a
    [1,512] PSUM accumulator - cheapest engine at 0.42ns/row), but the
    reciprocal broadcast is a 1-partition-contraction PE matmul
    (ones[1,128].T @ den[1,512] -> [128,512] PSUM, 213ns) instead of
    v1's SBUF->DRAM->SBUF round trip.
  - o-projection interleaved one query-chunk behind attention; o written
    as bf16 partials (host sums in fp32).
  - RMS+RoPE: Square/Sqrt/q*rms on ACT, rope muls on DVE (fp32 with one
    final bf16 rounding), transposed q/k evicted from PSUM by DVE.
"""

import sys

sys.path.insert(0, "/opt/trn_rl_repo")

import numpy as np

import concourse.bass as bass
import concourse.tile as tile
from concourse import bacc, mybir
from concourse.bass_utils import run_bass_kernel_spmd
from concourse.masks import make_identity

F32 = mybir.dt.float32
F32R = mybir.dt.float32r
BF = mybir.dt.bfloat16
AF = mybir.ActivationFunctionType

B = 2
S = 2048
D = 2048
NH = 16
NKV = 4
HD = 128
G = NH // NKV  # 4 query heads per KV head / per core
DQ = G * HD  # 512 query dims per core
EPS = 1e-6
ROPE_BASE = 10000.0

NT = S // 128  # 16 sequence tiles
ND = D // 128  # 16 contraction slices
QC = 4  # query chunks of 512
KT = S // 128  # 16 key tiles

_cached_nc = None
last_results = None  # BassKernelResults of the most recent run (for test.py)


def _build_program():
    nc = bacc.Bacc("TRN2", target_bir_lowering=False, debug=False)

    xt = nc.dram_tensor("xt", [128, NT * ND * 128], BF, kind="ExternalInput").ap()
    wqkv = nc.dram_tensor(
        "wqkv", [128, ND * (DQ + 2 * HD)], BF, kind="ExternalInput"
    ).ap()
    wo = nc.dram_tensor("wo", [128, G * D], BF, kind="ExternalInput").ap()
    cq = nc.dram_tensor("cq", [128, NT * HD], BF, kind="ExternalInput").ap()
    sq = nc.dram_tensor("sq", [128, NT * HD], BF, kind="ExternalInput").ap()
    ck = nc.dram_tensor("ck", [128, NT * HD], BF, kind="ExternalInput").ap()
    sk = nc.dram_tensor("sk", [128, NT * HD], BF, kind="ExternalInput").ap()
    ones = nc.dram_tensor("ones", [128, 128], F32R, kind="ExternalInput").ap()
    o = nc.dram_tensor("o", [S, D], BF, kind="ExternalOutput").ap()

    def xcol_src(st):
        return bass.AP(
            tensor=xt.tensor,
            offset=st * ND * 128,
            ap=[[NT * ND * 128, 128], [1, ND * 128]],
        )

    with tile.TileContext(nc) as tc:
        from contextlib import ExitStack

        with ExitStack() as ctx:
            ctx.enter_context(nc.allow_low_precision(reason="bf16 attention"))
            persist = ctx.enter_context(tc.tile_pool(name="persist", bufs=1))

            # persistent SBUF tensors
            wqkv_sb = persist.tile([128, ND, DQ + 2 * HD], BF, tag="wqkv")
            cq_sb = persist.tile([128, NT, HD], BF, tag="cq")
            sq_sb = persist.tile([128, NT, HD], BF, tag="sq")
            ck_sb = persist.tile([128, NT, HD], BF, tag="ck")
            sk_sb = persist.tile([128, NT, HD], BF, tag="sk")
            qt_all = persist.tile([128, G, S], BF, tag="qt")
            kt_sb = persist.tile([128, S], BF, tag="kt")
            v_sb = persist.tile([128, KT, HD], F32R, tag="v")
            outt_all = persist.tile([128, G, S], BF, tag="outt")
            ident = persist.tile([128, 128], F32, tag="ident")
            eps_sb = persist.tile([128, 1], F32, tag="eps")
            ones_sb = persist.tile([128, 128], F32R, tag="ones")

            # ---------------- Phase 1: projections + RMS + RoPE ----------------
            with ExitStack() as p1:
                p1.enter_context(nc.named_scope("p1_proj"))
                xpool = p1.enter_context(tc.tile_pool(name="xcol", bufs=3))
                rope = p1.enter_context(tc.tile_pool(name="rope", bufs=3))
                small = p1.enter_context(tc.tile_pool(name="small", bufs=3))
                ps1 = p1.enter_context(tc.tile_pool(name="ps1", bufs=3, space="PSUM"))
                pst = p1.enter_context(tc.tile_pool(name="pst", bufs=1, space="PSUM"))

                # prologue DMAs, most-urgent first: x tile 0, then Wq in 4
                # chunks (PE consumes ds-major), Wkv, rope tables.
                xcols = [None] * NT
                for j in range(2):
                    xcols[j] = xpool.tile(
                        [128, ND, 128], BF, name=f"xcol{j}", tag="xcol"
                    )
                    nc.sync.dma_start(xcols[j][:], xcol_src(j))
                W = DQ + 2 * HD
                for i in range(4):
                    nc.sync.dma_start(
                        wqkv_sb[:, 4 * i : 4 * i + 4, :],
                        bass.AP(
                            tensor=wqkv.tensor,
                            offset=4 * i * W,
                            ap=[[ND * W, 128], [1, 4 * W]],
                        ),
                    )
                for t, t_sb in ((cq, cq_sb), (sq, sq_sb), (ck, ck_sb), (sk, sk_sb)):
                    nc.sync.dma_start(t_sb[:], t)
                nc.sync.dma_start(ones_sb[:], ones)
                make_identity(nc, ident[:])
                nc.vector.memset(eps_sb[:], EPS)

                for st in range(NT):
                    if xcols[st] is None:
                        xcols[st] = xpool.tile(
                            [128, ND, 128], BF, name=f"xcol{st}", tag="xcol"
                        )
                        nc.sync.dma_start(xcols[st][:], xcol_src(st))
                    if st + 2 < NT and xcols[st + 2] is None:
                        xcols[st + 2] = xpool.tile(
                            [128, ND, 128], BF, name=f"xcol{st + 2}", tag="xcol"
                        )
                        nc.sync.dma_start(xcols[st + 2][:], xcol_src(st + 2))
                    xcol = xcols[st]

                    q_ps = ps1.tile([128, DQ], F32, tag="q_ps")
                    kv_ps = ps1.tile([128, 2 * HD], F32, tag="kv_ps")
                    for ds in range(ND):
                        nc.tensor.matmul(
                            q_ps[:],
                            xcol[:, ds, :],
                            wqkv_sb[:, ds, 0:DQ],
                            start=(ds == 0),
                            stop=(ds == ND - 1),
                        )
                        nc.tensor.matmul(
                            kv_ps[:],
                            xcol[:, ds, :],
                            wqkv_sb[:, ds, DQ : DQ + 2 * HD],
                            start=(ds == 0),
                            stop=(ds == ND - 1),
                        )

                    # v: straight copy to natural layout
                    nc.scalar.copy(v_sb[:, st, :], kv_ps[:, HD : 2 * HD])

                    # RMS norms: sum-of-squares per head chunk, one Sqrt, recip
                    ssq = small.tile([128, G + 1], F32, tag="ssq")
                    for hc in range(G + 1):
                        src = (
                            q_ps[:, hc * HD : (hc + 1) * HD]
                            if hc < G
                            else kv_ps[:, 0:HD]
                        )
                        sqv = small.tile([128, HD], F32, tag="sqv")
                        nc.scalar.activation(
                            sqv[:], src, AF.Square, accum_out=ssq[:, hc : hc + 1]
                        )
                    rms = small.tile([128, G + 1], F32, tag="rms")
                    nc.scalar.activation(
                        rms[:], ssq[:], AF.Sqrt, bias=eps_sb[:], scale=1.0 / HD
                    )
                    nc.vector.reciprocal(rms[:], rms[:])

                    trq = pst.tile([128, G + 1, 128], F32, tag="trq")
                    for hc in range(G + 1):
                        if hc < G:
                            src = q_ps[:, hc * HD : (hc + 1) * HD]
                            cos_t, sin_t = cq_sb[:, st, :], sq_sb[:, st, :]
                        else:
                            src = kv_ps[:, 0:HD]
                            cos_t, sin_t = ck_sb[:, st, :], sk_sb[:, st, :]

                        qh = rope.tile([128, HD], F32, tag="qh")
                        nc.scalar.mul(qh[:], src, rms[:, hc : hc + 1])

                        # rotate-half view: qh[p, (f+64) % 128]
                        rot = bass.AP(
                            tensor=qh[:].tensor,
                            offset=qh[:].offset + 64,
                            ap=[qh[:].ap[0], [-64, 2], [1, 64]],
                        )
                        t1 = rope.tile([128, HD], F32, tag="t1")
                        t2 = rope.tile([128, HD], F32, tag="t2")
                        nc.vector.tensor_mul(t1[:], qh[:], cos_t)
                        nc.vector.tensor_mul(
                            t2[:].rearrange("p (a b) -> p a b", a=2),
                            rot,
                            sin_t.rearrange("p (a b) -> p a b", a=2),
                        )
                        qr = rope.tile([128, HD], F32, tag="qr")
                        nc.vector.tensor_add(qr[:], t1[:], t2[:])

                        nc.tensor.transpose(trq[:, hc, :], qr[:], ident[:])
                    nc.vector.tensor_copy(
                        qt_all[:, :, st * 128 : (st + 1) * 128], trq[:, 0:G, :]
                    )
                    nc.vector.tensor_copy(
                        kt_sb[:, st * 128 : (st + 1) * 128], trq[:, G, :]
                    )

            # wo prefetch: phase-1 pools are released; load now so phase 3
            # never waits on this DMA.
            wo_sb = persist.tile([128, G, D], BF, tag="wo")
            nc.sync.dma_start(wo_sb[:], wo)

            # ---------------- Phase 2+3: attention with interleaved o-proj ----
            with ExitStack() as p2:
                p2.enter_context(nc.named_scope("p2_attn"))
                epool = p2.enter_context(tc.tile_pool(name="exp", bufs=2))
                dpool = p2.enter_context(tc.tile_pool(name="den", bufs=2))
                opool = p2.enter_context(tc.tile_pool(name="osb", bufs=3))
                ps_s = p2.enter_context(tc.tile_pool(name="ps_s", bufs=3, space="PSUM"))
                ps_av = p2.enter_context(
                    tc.tile_pool(name="ps_av", bufs=2, space="PSUM")
                )
                ps_dn = p2.enter_context(
                    tc.tile_pool(name="ps_dn", bufs=1, space="PSUM")
                )
                ps_o = p2.enter_context(tc.tile_pool(name="ps_o", bufs=2, space="PSUM"))

                def attn(qc):
                    qsl = slice(qc * 512, (qc + 1) * 512)
                    for h in range(G):
                        # Denominator strategy: PE per-kt ones-matmuls are the
                        # cheapest (0.42ns/row) but PE is the global
                        # bottleneck, so most blocks sum the exp tiles with an
                        # in-place pairwise tree (level 1 on the idle GpSimd,
                        # levels 2-4 on DVE) and do a single 213ns broadcast
                        # ones-matmul on the reduced tile.
                        pe_den = (qc, h) in ((0, 0), (2, 0))
                        exp_sb = epool.tile([128, KT, 512], F32R, tag="exp")
                        av_ps = ps_av.tile([128, 512], F32, tag="av")
                        den_ps = ps_dn.tile([128, 512], F32, tag="den")
                        for kt in range(KT):
                            s_ps = ps_s.tile([128, 512], F32, tag="s")
                            nc.tensor.matmul(
                                s_ps[:],
                                kt_sb[:, kt * 128 : (kt + 1) * 128],
                                qt_all[:, h, qsl],
                                start=True,
                                stop=True,
                            )
                            nc.scalar.activation(exp_sb[:, kt, :], s_ps[:], AF.Exp)
                            if pe_den:
                                nc.tensor.matmul(
                                    den_ps[:],
                                    ones_sb[:],
                                    exp_sb[:, kt, :],
                                    start=(kt == 0),
                                    stop=(kt == KT - 1),
                                )
                            nc.tensor.matmul(
                                av_ps[:],
                                v_sb[:, kt, :],
                                exp_sb[:, kt, :],
                                start=(kt == 0),
                                stop=(kt == KT - 1),
                            )

                        if not pe_den:
                            nc.gpsimd.tensor_add(
                                exp_sb[:, 0:8, :],
                                exp_sb[:, 0:8, :],
                                exp_sb[:, 8:16, :],
                            )
                            w = 4
                            while w >= 1:
                                nc.vector.tensor_add(
                                    exp_sb[:, 0:w, :],
                                    exp_sb[:, 0:w, :],
                                    exp_sb[:, w : 2 * w, :],
                                )
                                w //= 2
                            nc.tensor.matmul(
                                den_ps[:],
                                ones_sb[:],
                                exp_sb[:, 0, :],
                                start=True,
                                stop=True,
                            )
                        rbc = dpool.tile([128, 512], F32, tag="rbc")
                        nc.vector.reciprocal_approx_fast(rbc[:], den_ps[:])
                        nc.vector.tensor_mul(outt_all[:, h, qsl], av_ps[:], rbc[:])

                def oproj(qc):
                    for st in range(4 * qc, 4 * qc + 4):
                        o_sb = opool.tile([128, 4, 512], BF, tag="o_sb")
                        for dc in range(4):
                            op_ps = ps_o.tile([128, 512], F32, tag="op")
                            for h in range(G):
                                nc.tensor.matmul(
                                    op_ps[:],
                                    outt_all[:, h, st * 128 : (st + 1) * 128],
                                    wo_sb[:, h, dc * 512 : (dc + 1) * 512],
                                    start=(h == 0),
                                    stop=(h == G - 1),
                                )
                            nc.vector.tensor_copy(o_sb[:, dc, :], op_ps[:])
                            if dc % 2 == 1:
                                nc.sync.dma_start(
                                    bass.AP(
                                        tensor=o.tensor,
                                        offset=st * 128 * D + (dc - 1) * 512,
                                        ap=[[D, 128], [1, 1024]],
                                    ),
                                    o_sb[:, dc - 1 : dc + 1, :].rearrange(
                                        "p a b -> p (a b)"
                                    ),
                                )

                attn(0)
                attn(1)
                oproj(0)
                attn(2)
                oproj(1)
                attn(3)
                oproj(2)
                oproj(3)

    nc.compile()
    return nc


def _rope_tables(qw, kw):
    """Folded cos/sin tables. RoPE rotation with rotate-half; per-head RMS
    norm weight w and the attention scale sc are folded in:
      out[d] = qhat[d]*w[d]*cos[d]*sc + qhat[(d+64)%128]*(sgn)*w[(d+64)%128]*sin[d]*sc
    where sgn = -1 for d < 64 (rotate-half negates the upper half moved down).
    """
    inv_freq = 1.0 / (ROPE_BASE ** (np.arange(0, HD, 2, dtype=np.float32) / HD))
    t = np.arange(S, dtype=np.float32)
    freqs = np.outer(t, inv_freq).astype(np.float32)  # [S, 64]
    emb = np.concatenate([freqs, freqs], axis=1)  # [S, 128]
    cos = np.cos(emb).astype(np.float32)
    sin = np.sin(emb).astype(np.float32)

    sgn = np.where(np.arange(HD) < 64, np.float32(-1.0), np.float32(1.0))
    wshift_q = np.roll(qw, -64)  # w[(d+64)%128]
    wshift_k = np.roll(kw, -64)
    sc = np.float32(1.0 / np.sqrt(HD))
    cq = cos * qw[None, :] * sc
    sq_ = sin * (sgn * wshift_q)[None, :] * sc
    ck = cos * kw[None, :]
    sk_ = sin * (sgn * wshift_k)[None, :]
    return cq, sq_, ck, sk_


def _sw_rows(m, nblk):
    """[nblk*128, L] row-major -> [128, nblk*L] partition-major (p = row % 128
    within block, line = concatenated blocks)."""
    L = m.shape[1]
    return np.ascontiguousarray(
        m.reshape(nblk, 128, L).transpose(1, 0, 2).reshape(128, nblk * L)
    )


def kernel(x, Wq, Wk, Wv, Wo, q_norm_w, k_norm_w):
    global _cached_nc, last_results
    import ml_dtypes

    bf16 = ml_dtypes.bfloat16
    x = np.asarray(x, dtype=np.float32)
    Wq = np.asarray(Wq, dtype=np.float32)
    Wk = np.asarray(Wk, dtype=np.float32)
    Wv = np.asarray(Wv, dtype=np.float32)
    Wo = np.asarray(Wo, dtype=np.float32)
    qw = np.asarray(q_norm_w, dtype=np.float32)
    kw = np.asarray(k_norm_w, dtype=np.float32)

    if _cached_nc is None:
        _cached_nc = _build_program()
    nc = _cached_nc

    cqt, sqt, ckt, skt = _rope_tables(qw, kw)

    in_maps = []
    for c in range(8):
        b, g = divmod(c, 4)
        # x.T pre-swizzled: xt_sw[p, st, ds, f] = x[b][st*128+f, ds*128+p]
        xt_sw = np.ascontiguousarray(
            x[b]
            .reshape(NT, 128, ND, 128)
            .transpose(3, 0, 2, 1)
            .reshape(128, NT * ND * 128)
            .astype(bf16)
        )
        wqkv_full = np.concatenate(
            [
                Wq[:, g * DQ : (g + 1) * DQ],
                Wk[:, g * HD : (g + 1) * HD],
                Wv[:, g * HD : (g + 1) * HD],
            ],
            axis=1,
        )
        in_maps.append(
            {
                "xt": xt_sw,
                "wqkv": _sw_rows(wqkv_full, ND).astype(bf16),
                "wo": _sw_rows(Wo[g * DQ : (g + 1) * DQ, :], G).astype(bf16),
                "cq": _sw_rows(cqt, NT).astype(bf16),
                "sq": _sw_rows(sqt, NT).astype(bf16),
                "ck": _sw_rows(ckt, NT).astype(bf16),
                "sk": _sw_rows(skt, NT).astype(bf16),
                "ones": np.ones((128, 128), dtype=np.float32),
            }
        )

    last_results = run_bass_kernel_spmd(nc, in_maps, core_ids=list(range(8)))

    out = np.zeros((B, S, D), dtype=np.float32)
    for c in range(8):
        b = c // 4
        out[b] += np.asarray(last_results.results[c]["o"], dtype=np.float32)
    return out


# revision 12
# speedup vs baseline: 1.1747x; 1.1747x over previous
"""Grouped-Query Attention forward pass on 8 Trainium2 NeuronCores.

Sharding: 2-way data parallel over batch x 4-way tensor parallel over KV
head groups. Core c = 4*b + g handles batch b and KV group g (4 query
heads + 1 KV head). Each core computes a partial o-projection output
(its head group's contribution, full [S, D]); the host sums the 4
partials per batch.

v3 design (vs 424us v1 baseline):
  - bf16 matmuls for projections / scores / o-proj (same 1 cycle/row PE
    rate as float32r, halves DMA+SBUF).  exp and AV stay float32r: the
    ACT engine writes bf16 at half rate, so bf16 exp tiles are a loss.
  - All DRAM operands host-pre-swizzled to [128-partition, long-line]
    layouts (4-16KB contiguous per partition per DMA descriptor); v1
    used 256-512B lines and spent 46us of prologue on ~52k descriptors.
    First x tile + chunked Wq are issued before everything else so PE
    starts at ~6us.
  - Softmax denominator stays on PE (per-key-tile ones-matmul into a
    [1,512] PSUM accumulator - cheapest engine at 0.42ns/row), but the
    reciprocal broadcast is a 1-partition-contraction PE matmul
    (ones[1,128].T @ den[1,512] -> [128,512] PSUM, 213ns) instead of
    v1's SBUF->DRAM->SBUF round trip.
  - o-projection interleaved one query-chunk behind attention; o written
    as bf16 partials (host sums in fp32).
  - RMS+RoPE: Square/Sqrt/q*rms on ACT, rope muls on DVE (fp32 with one
    final bf16 rounding), transposed q/k evicted from PSUM by DVE.
"""

import sys

sys.path.insert(0, "/opt/trn_rl_repo")

import numpy as np

import concourse.bass as bass
import concourse.tile as tile
from concourse import bacc, mybir
from concourse.bass_utils import run_bass_kernel_spmd
from concourse.masks import make_identity

F32 = mybir.dt.float32
F32R = mybir.dt.float32r
BF = mybir.dt.bfloat16
AF = mybir.ActivationFunctionType

B = 2
S = 2048
D = 2048
NH = 16
NKV = 4
HD = 128
G = NH // NKV  # 4 query heads per KV head / per core
DQ = G * HD  # 512 query dims per core
EPS = 1e-6
ROPE_BASE = 10000.0

NT = S // 128  # 16 sequence tiles
ND = D // 128  # 16 contraction slices
QC = 4  # query chunks of 512
KT = S // 128  # 16 key tiles

_cached_nc = None
last_results = None  # BassKernelResults of the most recent run (for test.py)


def _build_program():
    nc = bacc.Bacc("TRN2", target_bir_lowering=False, debug=False)

    xt = nc.dram_tensor("xt", [128, NT * ND * 128], BF, kind="ExternalInput").ap()
    wqkv = nc.dram_tensor(
        "wqkv", [128, ND * (DQ + 2 * HD)], BF, kind="ExternalInput"
    ).ap()
    wo = nc.dram_tensor("wo", [128, G * D], BF, kind="ExternalInput").ap()
    cq = nc.dram_tensor("cq", [128, NT * HD], BF, kind="ExternalInput").ap()
    sq = nc.dram_tensor("sq", [128, NT * HD], BF, kind="ExternalInput").ap()
    ck = nc.dram_tensor("ck", [128, NT * HD], BF, kind="ExternalInput").ap()
    sk = nc.dram_tensor("sk", [128, NT * HD], BF, kind="ExternalInput").ap()
    ones = nc.dram_tensor("ones", [128, 128], F32R, kind="ExternalInput").ap()
    o = nc.dram_tensor("o", [S, D], BF, kind="ExternalOutput").ap()

    def xcol_src(st):
        return bass.AP(
            tensor=xt.tensor,
            offset=st * ND * 128,
            ap=[[NT * ND * 128, 128], [1, ND * 128]],
        )

    with tile.TileContext(nc) as tc:
        from contextlib import ExitStack

        with ExitStack() as ctx:
            ctx.enter_context(nc.allow_low_precision(reason="bf16 attention"))
            persist = ctx.enter_context(tc.tile_pool(name="persist", bufs=1))

            # persistent SBUF tensors
            wqkv_sb = persist.tile([128, ND, DQ + 2 * HD], BF, tag="wqkv")
            cq_sb = persist.tile([128, NT, HD], BF, tag="cq")
            sq_sb = persist.tile([128, NT, HD], BF, tag="sq")
            ck_sb = persist.tile([128, NT, HD], BF, tag="ck")
            sk_sb = persist.tile([128, NT, HD], BF, tag="sk")
            qt_all = persist.tile([128, G, S], BF, tag="qt")
            kt_sb = persist.tile([128, S], BF, tag="kt")
            v_sb = persist.tile([128, KT, HD], F32R, tag="v")
            outt_all = persist.tile([128, G, S], BF, tag="outt")
            ident = persist.tile([128, 128], F32, tag="ident")
            eps_sb = persist.tile([128, 1], F32, tag="eps")
            ones_sb = persist.tile([128, 128], F32R, tag="ones")

            # ---------------- Phase 1: projections + RMS + RoPE ----------------
            with ExitStack() as p1:
                p1.enter_context(nc.named_scope("p1_proj"))
                xpool = p1.enter_context(tc.tile_pool(name="xcol", bufs=3))
                rope = p1.enter_context(tc.tile_pool(name="rope", bufs=3))
                small = p1.enter_context(tc.tile_pool(name="small", bufs=3))
                ps1 = p1.enter_context(tc.tile_pool(name="ps1", bufs=3, space="PSUM"))
                pst = p1.enter_context(tc.tile_pool(name="pst", bufs=1, space="PSUM"))

                # prologue DMAs, most-urgent first: x tile 0, then Wq in 4
                # chunks (PE consumes ds-major), Wkv, rope tables.
                xcols = [None] * NT
                for j in range(2):
                    xcols[j] = xpool.tile(
                        [128, ND, 128], BF, name=f"xcol{j}", tag="xcol"
                    )
                    nc.sync.dma_start(xcols[j][:], xcol_src(j))
                W = DQ + 2 * HD
                for i in range(4):
                    nc.sync.dma_start(
                        wqkv_sb[:, 4 * i : 4 * i + 4, :],
                        bass.AP(
                            tensor=wqkv.tensor,
                            offset=4 * i * W,
                            ap=[[ND * W, 128], [1, 4 * W]],
                        ),
                    )
                for t, t_sb in ((cq, cq_sb), (sq, sq_sb), (ck, ck_sb), (sk, sk_sb)):
                    nc.sync.dma_start(t_sb[:], t)
                nc.sync.dma_start(ones_sb[:], ones)
                make_identity(nc, ident[:])
                nc.vector.memset(eps_sb[:], EPS)

                for st in range(NT):
                    if xcols[st] is None:
                        xcols[st] = xpool.tile(
                            [128, ND, 128], BF, name=f"xcol{st}", tag="xcol"
                        )
                        nc.sync.dma_start(xcols[st][:], xcol_src(st))
                    if st + 2 < NT and xcols[st + 2] is None:
                        xcols[st + 2] = xpool.tile(
                            [128, ND, 128], BF, name=f"xcol{st + 2}", tag="xcol"
                        )
                        nc.sync.dma_start(xcols[st + 2][:], xcol_src(st + 2))
                    xcol = xcols[st]

                    q_ps = ps1.tile([128, DQ], F32, tag="q_ps")
                    kv_ps = ps1.tile([128, 2 * HD], F32, tag="kv_ps")
                    for ds in range(ND):
                        nc.tensor.matmul(
                            q_ps[:],
                            xcol[:, ds, :],
                            wqkv_sb[:, ds, 0:DQ],
                            start=(ds == 0),
                            stop=(ds == ND - 1),
                        )
                        nc.tensor.matmul(
                            kv_ps[:],
                            xcol[:, ds, :],
                            wqkv_sb[:, ds, DQ : DQ + 2 * HD],
                            start=(ds == 0),
                            stop=(ds == ND - 1),
                        )

                    # v: straight copy to natural layout
                    nc.scalar.copy(v_sb[:, st, :], kv_ps[:, HD : 2 * HD])

                    # RMS norms: sum-of-squares per head chunk, one Sqrt, recip
                    ssq = small.tile([128, G + 1], F32, tag="ssq")
                    for hc in range(G + 1):
                        src = (
                            q_ps[:, hc * HD : (hc + 1) * HD]
                            if hc < G
                            else kv_ps[:, 0:HD]
                        )
                        sqv = small.tile([128, HD], F32, tag="sqv")
                        nc.scalar.activation(
                            sqv[:], src, AF.Square, accum_out=ssq[:, hc : hc + 1]
                        )
                    rms = small.tile([128, G + 1], F32, tag="rms")
                    nc.scalar.activation(
                        rms[:], ssq[:], AF.Sqrt, bias=eps_sb[:], scale=1.0 / HD
                    )
                    nc.vector.reciprocal(rms[:], rms[:])

                    trq = pst.tile([128, G + 1, 128], F32, tag="trq")
                    for hc in range(G + 1):
                        if hc < G:
                            src = q_ps[:, hc * HD : (hc + 1) * HD]
                            cos_t, sin_t = cq_sb[:, st, :], sq_sb[:, st, :]
                        else:
                            src = kv_ps[:, 0:HD]
                            cos_t, sin_t = ck_sb[:, st, :], sk_sb[:, st, :]

                        qh = rope.tile([128, HD], F32, tag="qh")
                        nc.scalar.mul(qh[:], src, rms[:, hc : hc + 1])

                        # rotate-half view: qh[p, (f+64) % 128]
                        rot = bass.AP(
                            tensor=qh[:].tensor,
                            offset=qh[:].offset + 64,
                            ap=[qh[:].ap[0], [-64, 2], [1, 64]],
                        )
                        t1 = rope.tile([128, HD], F32, tag="t1")
                        t2 = rope.tile([128, HD], F32, tag="t2")
                        nc.vector.tensor_mul(t1[:], qh[:], cos_t)
                        nc.vector.tensor_mul(
                            t2[:].rearrange("p (a b) -> p a b", a=2),
                            rot,
                            sin_t.rearrange("p (a b) -> p a b", a=2),
                        )
                        qr = rope.tile([128, HD], F32, tag="qr")
                        nc.vector.tensor_add(qr[:], t1[:], t2[:])

                        nc.tensor.transpose(trq[:, hc, :], qr[:], ident[:])
                    nc.vector.tensor_copy(
                        qt_all[:, :, st * 128 : (st + 1) * 128], trq[:, 0:G, :]
                    )
                    nc.vector.tensor_copy(
                        kt_sb[:, st * 128 : (st + 1) * 128], trq[:, G, :]
                    )

            # wo prefetch: phase-1 pools are released; load now so phase 3
            # never waits on this DMA.
            wo_sb = persist.tile([128, G, D], BF, tag="wo")
            nc.sync.dma_start(wo_sb[:], wo)

            # ---------------- Phase 2+3: attention with interleaved o-proj ----
            with ExitStack() as p2:
                p2.enter_context(nc.named_scope("p2_attn"))
                epool = p2.enter_context(tc.tile_pool(name="exp", bufs=2))
                dpool = p2.enter_context(tc.tile_pool(name="den", bufs=2))
                opool = p2.enter_context(tc.tile_pool(name="osb", bufs=3))
                ps_s = p2.enter_context(tc.tile_pool(name="ps_s", bufs=3, space="PSUM"))
                ps_av = p2.enter_context(
                    tc.tile_pool(name="ps_av", bufs=2, space="PSUM")
                )
                ps_dn = p2.enter_context(
                    tc.tile_pool(name="ps_dn", bufs=1, space="PSUM")
                )
                ps_o = p2.enter_context(tc.tile_pool(name="ps_o", bufs=2, space="PSUM"))

                def attn(qc):
                    qsl = slice(qc * 512, (qc + 1) * 512)
                    for h in range(G):
                        # Denominator strategy: PE per-kt ones-matmuls are the
                        # cheapest (0.42ns/row) but PE is the global
                        # bottleneck, so most blocks sum the exp tiles with an
                        # in-place pairwise tree (level 1 on the idle GpSimd,
                        # levels 2-4 on DVE) and do a single 213ns broadcast
                        # ones-matmul on the reduced tile.
                        pe_den = h == 0
                        exp_sb = epool.tile([128, KT, 512], F32R, tag="exp")
                        av_ps = ps_av.tile([128, 512], F32, tag="av")
                        den_ps = ps_dn.tile([128, 512], F32, tag="den")
                        for kt in range(KT):
                            s_ps = ps_s.tile([128, 512], F32, tag="s")
                            nc.tensor.matmul(
                                s_ps[:],
                                kt_sb[:, kt * 128 : (kt + 1) * 128],
                                qt_all[:, h, qsl],
                                start=True,
                                stop=True,
                            )
                            nc.scalar.activation(exp_sb[:, kt, :], s_ps[:], AF.Exp)
                            if pe_den:
                                nc.tensor.matmul(
                                    den_ps[:],
                                    ones_sb[:],
                                    exp_sb[:, kt, :],
                                    start=(kt == 0),
                                    stop=(kt == KT - 1),
                                )
                            nc.tensor.matmul(
                                av_ps[:],
                                v_sb[:, kt, :],
                                exp_sb[:, kt, :],
                                start=(kt == 0),
                                stop=(kt == KT - 1),
                            )

                        if not pe_den:
                            w = 8
                            while w >= 1:
                                nc.vector.tensor_add(
                                    exp_sb[:, 0:w, :],
                                    exp_sb[:, 0:w, :],
                                    exp_sb[:, w : 2 * w, :],
                                )
                                w //= 2
                            nc.tensor.matmul(
                                den_ps[:],
                                ones_sb[:],
                                exp_sb[:, 0, :],
                                start=True,
                                stop=True,
                            )
                        rbc = dpool.tile([128, 512], F32, tag="rbc")
                        nc.vector.reciprocal_approx_fast(rbc[:], den_ps[:])
                        nc.vector.tensor_mul(outt_all[:, h, qsl], av_ps[:], rbc[:])

                def oproj(qc):
                    for st in range(4 * qc, 4 * qc + 4):
                        o_sb = opool.tile([128, 4, 512], BF, tag="o_sb")
                        for dc in range(4):
                            op_ps = ps_o.tile([128, 512], F32, tag="op")
                            for h in range(G):
                                nc.tensor.matmul(
                                    op_ps[:],
                                    outt_all[:, h, st * 128 : (st + 1) * 128],
                                    wo_sb[:, h, dc * 512 : (dc + 1) * 512],
                                    start=(h == 0),
                                    stop=(h == G - 1),
                                )
                            nc.vector.tensor_copy(o_sb[:, dc, :], op_ps[:])
                            if dc % 2 == 1:
                                nc.sync.dma_start(
                                    bass.AP(
                                        tensor=o.tensor,
                                        offset=st * 128 * D + (dc - 1) * 512,
                                        ap=[[D, 128], [1, 1024]],
                                    ),
                                    o_sb[:, dc - 1 : dc + 1, :].rearrange(
                                        "p a b -> p (a b)"
                                    ),
                                )

                attn(0)
                attn(1)
                oproj(0)
                attn(2)
                oproj(1)
                attn(3)
                oproj(2)
                oproj(3)

    nc.compile()
    return nc


def _rope_tables(qw, kw):
    """Folded cos/sin tables. RoPE rotation with rotate-half; per-head RMS
    norm weight w and the attention scale sc are folded in:
      out[d] = qhat[d]*w[d]*cos[d]*sc + qhat[(d+64)%128]*(sgn)*w[(d+64)%128]*sin[d]*sc
    where sgn = -1 for d < 64 (rotate-half negates the upper half moved down).
    """
    inv_freq = 1.0 / (ROPE_BASE ** (np.arange(0, HD, 2, dtype=np.float32) / HD))
    t = np.arange(S, dtype=np.float32)
    freqs = np.outer(t, inv_freq).astype(np.float32)  # [S, 64]
    emb = np.concatenate([freqs, freqs], axis=1)  # [S, 128]
    cos = np.cos(emb).astype(np.float32)
    sin = np.sin(emb).astype(np.float32)

    sgn = np.where(np.arange(HD) < 64, np.float32(-1.0), np.float32(1.0))
    wshift_q = np.roll(qw, -64)  # w[(d+64)%128]
    wshift_k = np.roll(kw, -64)
    sc = np.float32(1.0 / np.sqrt(HD))
    cq = cos * qw[None, :] * sc
    sq_ = sin * (sgn * wshift_q)[None, :] * sc
    ck = cos * kw[None, :]
    sk_ = sin * (sgn * wshift_k)[None, :]
    return cq, sq_, ck, sk_


def _sw_rows(m, nblk):
    """[nblk*128, L] row-major -> [128, nblk*L] partition-major (p = row % 128
    within block, line = concatenated blocks)."""
    L = m.shape[1]
    return np.ascontiguousarray(
        m.reshape(nblk, 128, L).transpose(1, 0, 2).reshape(128, nblk * L)
    )


def kernel(x, Wq, Wk, Wv, Wo, q_norm_w, k_norm_w):
    global _cached_nc, last_results
    import ml_dtypes

    bf16 = ml_dtypes.bfloat16
    x = np.asarray(x, dtype=np.float32)
    Wq = np.asarray(Wq, dtype=np.float32)
    Wk = np.asarray(Wk, dtype=np.float32)
    Wv = np.asarray(Wv, dtype=np.float32)
    Wo = np.asarray(Wo, dtype=np.float32)
    qw = np.asarray(q_norm_w, dtype=np.float32)
    kw = np.asarray(k_norm_w, dtype=np.float32)

    if _cached_nc is None:
        _cached_nc = _build_program()
    nc = _cached_nc

    cqt, sqt, ckt, skt = _rope_tables(qw, kw)

    in_maps = []
    for c in range(8):
        b, g = divmod(c, 4)
        # x.T pre-swizzled: xt_sw[p, st, ds, f] = x[b][st*128+f, ds*128+p]
        xt_sw = np.ascontiguousarray(
            x[b]
            .reshape(NT, 128, ND, 128)
            .transpose(3, 0, 2, 1)
            .reshape(128, NT * ND * 128)
            .astype(bf16)
        )
        wqkv_full = np.concatenate(
            [
                Wq[:, g * DQ : (g + 1) * DQ],
                Wk[:, g * HD : (g + 1) * HD],
                Wv[:, g * HD : (g + 1) * HD],
            ],
            axis=1,
        )
        in_maps.append(
            {
                "xt": xt_sw,
                "wqkv": _sw_rows(wqkv_full, ND).astype(bf16),
                "wo": _sw_rows(Wo[g * DQ : (g + 1) * DQ, :], G).astype(bf16),
                "cq": _sw_rows(cqt, NT).astype(bf16),
                "sq": _sw_rows(sqt, NT).astype(bf16),
                "ck": _sw_rows(ckt, NT).astype(bf16),
                "sk": _sw_rows(skt, NT).astype(bf16),
                "ones": np.ones((128, 128), dtype=np.float32),
            }
        )

    last_results = run_bass_kernel_spmd(nc, in_maps, core_ids=list(range(8)))

    out = np.zeros((B, S, D), dtype=np.float32)
    for c in range(8):
        b = c // 4
        out[b] += np.asarray(last_results.results[c]["o"], dtype=np.float32)
    return out


# revision 14
# speedup vs baseline: 1.1828x; 1.0069x over previous
"""Grouped-Query Attention forward pass on 8 Trainium2 NeuronCores.

Sharding: 2-way data parallel over batch x 4-way tensor parallel over KV
head groups. Core c = 4*b + g handles batch b and KV group g (4 query
heads + 1 KV head). Each core computes a partial o-projection output
(its head group's contribution, full [S, D]); the host sums the 4
partials per batch.

v3 design (vs 424us v1 baseline):
  - bf16 matmuls for projections / scores / o-proj (same 1 cycle/row PE
    rate as float32r, halves DMA+SBUF).  exp and AV stay float32r: the
    ACT engine writes bf16 at half rate, so bf16 exp tiles are a loss.
  - All DRAM operands host-pre-swizzled to [128-partition, long-line]
    layouts (4-16KB contiguous per partition per DMA descriptor); v1
    used 256-512B lines and spent 46us of prologue on ~52k descriptors.
    First x tile + chunked Wq are issued before everything else so PE
    starts at ~6us.
  - Softmax denominator stays on PE (per-key-tile ones-matmul into a
    [1,512] PSUM accumulator - cheapest engine at 0.42ns/row), but the
    reciprocal broadcast is a 1-partition-contraction PE matmul
    (ones[1,128].T @ den[1,512] -> [128,512] PSUM, 213ns) instead of
    v1's SBUF->DRAM->SBUF round trip.
  - o-projection interleaved one query-chunk behind attention; o written
    as bf16 partials (host sums in fp32).
  - RMS+RoPE: Square/Sqrt/q*rms on ACT, rope muls on DVE (fp32 with one
    final bf16 rounding), transposed q/k evicted from PSUM by DVE.
"""

import sys

sys.path.insert(0, "/opt/trn_rl_repo")

import numpy as np

import concourse.bass as bass
import concourse.tile as tile
from concourse import bacc, mybir
from concourse.bass_utils import run_bass_kernel_spmd
from concourse.masks import make_identity

F32 = mybir.dt.float32
F32R = mybir.dt.float32r
BF = mybir.dt.bfloat16
AF = mybir.ActivationFunctionType

B = 2
S = 2048
D = 2048
NH = 16
NKV = 4
HD = 128
G = NH // NKV  # 4 query heads per KV head / per core
DQ = G * HD  # 512 query dims per core
EPS = 1e-6
ROPE_BASE = 10000.0

NT = S // 128  # 16 sequence tiles
ND = D // 128  # 16 contraction slices
QC = 4  # query chunks of 512
KT = S // 128  # 16 key tiles

_cached_nc = None
last_results = None  # BassKernelResults of the most recent run (for test.py)


def _build_program():
    nc = bacc.Bacc("TRN2", target_bir_lowering=False, debug=False)

    xt = nc.dram_tensor("xt", [128, NT * ND * 128], BF, kind="ExternalInput").ap()
    wqkv = nc.dram_tensor(
        "wqkv", [128, ND * (DQ + 2 * HD)], BF, kind="ExternalInput"
    ).ap()
    wo = nc.dram_tensor("wo", [128, G * D], BF, kind="ExternalInput").ap()
    cq = nc.dram_tensor("cq", [128, NT * HD], BF, kind="ExternalInput").ap()
    sq = nc.dram_tensor("sq", [128, NT * HD], BF, kind="ExternalInput").ap()
    ck = nc.dram_tensor("ck", [128, NT * HD], BF, kind="ExternalInput").ap()
    sk = nc.dram_tensor("sk", [128, NT * HD], BF, kind="ExternalInput").ap()
    ones = nc.dram_tensor("ones", [128, 128], F32R, kind="ExternalInput").ap()
    o = nc.dram_tensor("o", [S, D], BF, kind="ExternalOutput").ap()

    def xcol_src(st):
        return bass.AP(
            tensor=xt.tensor,
            offset=st * ND * 128,
            ap=[[NT * ND * 128, 128], [1, ND * 128]],
        )

    with tile.TileContext(nc) as tc:
        from contextlib import ExitStack

        with ExitStack() as ctx:
            ctx.enter_context(nc.allow_low_precision(reason="bf16 attention"))
            persist = ctx.enter_context(tc.tile_pool(name="persist", bufs=1))

            # persistent SBUF tensors
            wqkv_sb = persist.tile([128, ND, DQ + 2 * HD], BF, tag="wqkv")
            cq_sb = persist.tile([128, NT, HD], BF, tag="cq")
            sq_sb = persist.tile([128, NT, HD], BF, tag="sq")
            ck_sb = persist.tile([128, NT, HD], BF, tag="ck")
            sk_sb = persist.tile([128, NT, HD], BF, tag="sk")
            qt_all = persist.tile([128, G, S], BF, tag="qt")
            kt_sb = persist.tile([128, S], BF, tag="kt")
            v_sb = persist.tile([128, KT, HD], F32R, tag="v")
            outt_all = persist.tile([128, G, S], BF, tag="outt")
            ident = persist.tile([128, 128], F32, tag="ident")
            eps_sb = persist.tile([128, 1], F32, tag="eps")
            ones_sb = persist.tile([128, 128], F32R, tag="ones")

            # ---------------- Phase 1: projections + RMS + RoPE ----------------
            with ExitStack() as p1:
                p1.enter_context(nc.named_scope("p1_proj"))
                xpool = p1.enter_context(tc.tile_pool(name="xcol", bufs=3))
                rope = p1.enter_context(tc.tile_pool(name="rope", bufs=3))
                small = p1.enter_context(tc.tile_pool(name="small", bufs=3))
                ps1 = p1.enter_context(tc.tile_pool(name="ps1", bufs=3, space="PSUM"))
                pst = p1.enter_context(tc.tile_pool(name="pst", bufs=1, space="PSUM"))

                # prologue DMAs, most-urgent first: x tile 0 and the first
                # Wqkv chunk (the first matmul needs only these), then the
                # rest of the weights, x tile 1, rope tables.
                W = DQ + 2 * HD

                def wqkv_chunk(i):
                    nc.sync.dma_start(
                        wqkv_sb[:, 4 * i : 4 * i + 4, :],
                        bass.AP(
                            tensor=wqkv.tensor,
                            offset=4 * i * W,
                            ap=[[ND * W, 128], [1, 4 * W]],
                        ),
                    )

                xcols = [None] * NT
                xcols[0] = xpool.tile([128, ND, 128], BF, name="xcol0", tag="xcol")
                nc.sync.dma_start(xcols[0][:], xcol_src(0))
                wqkv_chunk(0)
                xcols[1] = xpool.tile([128, ND, 128], BF, name="xcol1", tag="xcol")
                nc.sync.dma_start(xcols[1][:], xcol_src(1))
                for i in range(1, 4):
                    wqkv_chunk(i)
                for t, t_sb in ((cq, cq_sb), (sq, sq_sb), (ck, ck_sb), (sk, sk_sb)):
                    nc.sync.dma_start(t_sb[:], t)
                nc.sync.dma_start(ones_sb[:], ones)
                make_identity(nc, ident[:])
                nc.vector.memset(eps_sb[:], EPS)

                for st in range(NT):
                    if xcols[st] is None:
                        xcols[st] = xpool.tile(
                            [128, ND, 128], BF, name=f"xcol{st}", tag="xcol"
                        )
                        nc.sync.dma_start(xcols[st][:], xcol_src(st))
                    if st + 2 < NT and xcols[st + 2] is None:
                        xcols[st + 2] = xpool.tile(
                            [128, ND, 128], BF, name=f"xcol{st + 2}", tag="xcol"
                        )
                        nc.sync.dma_start(xcols[st + 2][:], xcol_src(st + 2))
                    xcol = xcols[st]

                    q_ps = ps1.tile([128, DQ], F32, tag="q_ps")
                    kv_ps = ps1.tile([128, 2 * HD], F32, tag="kv_ps")
                    for ds in range(ND):
                        nc.tensor.matmul(
                            q_ps[:],
                            xcol[:, ds, :],
                            wqkv_sb[:, ds, 0:DQ],
                            start=(ds == 0),
                            stop=(ds == ND - 1),
                        )
                        nc.tensor.matmul(
                            kv_ps[:],
                            xcol[:, ds, :],
                            wqkv_sb[:, ds, DQ : DQ + 2 * HD],
                            start=(ds == 0),
                            stop=(ds == ND - 1),
                        )

                    # v: straight copy to natural layout
                    nc.scalar.copy(v_sb[:, st, :], kv_ps[:, HD : 2 * HD])

                    # RMS norms: sum-of-squares per head chunk, one Sqrt, recip
                    ssq = small.tile([128, G + 1], F32, tag="ssq")
                    for hc in range(G + 1):
                        src = (
                            q_ps[:, hc * HD : (hc + 1) * HD]
                            if hc < G
                            else kv_ps[:, 0:HD]
                        )
                        sqv = small.tile([128, HD], F32, tag="sqv")
                        nc.scalar.activation(
                            sqv[:], src, AF.Square, accum_out=ssq[:, hc : hc + 1]
                        )
                    rms = small.tile([128, G + 1], F32, tag="rms")
                    nc.scalar.activation(
                        rms[:], ssq[:], AF.Sqrt, bias=eps_sb[:], scale=1.0 / HD
                    )
                    nc.vector.reciprocal(rms[:], rms[:])

                    trq = pst.tile([128, G + 1, 128], F32, tag="trq")
                    for hc in range(G + 1):
                        if hc < G:
                            src = q_ps[:, hc * HD : (hc + 1) * HD]
                            cos_t, sin_t = cq_sb[:, st, :], sq_sb[:, st, :]
                        else:
                            src = kv_ps[:, 0:HD]
                            cos_t, sin_t = ck_sb[:, st, :], sk_sb[:, st, :]

                        qh = rope.tile([128, HD], F32, tag="qh")
                        nc.scalar.mul(qh[:], src, rms[:, hc : hc + 1])

                        # rotate-half view: qh[p, (f+64) % 128]
                        rot = bass.AP(
                            tensor=qh[:].tensor,
                            offset=qh[:].offset + 64,
                            ap=[qh[:].ap[0], [-64, 2], [1, 64]],
                        )
                        t1 = rope.tile([128, HD], F32, tag="t1")
                        t2 = rope.tile([128, HD], F32, tag="t2")
                        nc.vector.tensor_mul(t1[:], qh[:], cos_t)
                        nc.vector.tensor_mul(
                            t2[:].rearrange("p (a b) -> p a b", a=2),
                            rot,
                            sin_t.rearrange("p (a b) -> p a b", a=2),
                        )
                        qr = rope.tile([128, HD], F32, tag="qr")
                        nc.vector.tensor_add(qr[:], t1[:], t2[:])

                        nc.tensor.transpose(trq[:, hc, :], qr[:], ident[:])
                    nc.vector.tensor_copy(
                        qt_all[:, :, st * 128 : (st + 1) * 128], trq[:, 0:G, :]
                    )
                    nc.vector.tensor_copy(
                        kt_sb[:, st * 128 : (st + 1) * 128], trq[:, G, :]
                    )

            # wo prefetch: phase-1 pools are released; load now so phase 3
            # never waits on this DMA.
            wo_sb = persist.tile([128, G, D], BF, tag="wo")
            nc.sync.dma_start(wo_sb[:], wo)

            # ---------------- Phase 2+3: attention with interleaved o-proj ----
            with ExitStack() as p2:
                p2.enter_context(nc.named_scope("p2_attn"))
                epool = p2.enter_context(tc.tile_pool(name="exp", bufs=2))
                dpool = p2.enter_context(tc.tile_pool(name="den", bufs=2))
                opool = p2.enter_context(tc.tile_pool(name="osb", bufs=3))
                ps_s = p2.enter_context(tc.tile_pool(name="ps_s", bufs=3, space="PSUM"))
                ps_av = p2.enter_context(
                    tc.tile_pool(name="ps_av", bufs=2, space="PSUM")
                )
                ps_dn = p2.enter_context(
                    tc.tile_pool(name="ps_dn", bufs=1, space="PSUM")
                )
                ps_o = p2.enter_context(tc.tile_pool(name="ps_o", bufs=2, space="PSUM"))

                def attn(qc):
                    qsl = slice(qc * 512, (qc + 1) * 512)
                    for h in range(G):
                        # Denominator strategy: PE per-kt ones-matmuls are the
                        # cheapest (0.42ns/row) but PE is the global
                        # bottleneck, so most blocks sum the exp tiles with an
                        # in-place pairwise tree (level 1 on the idle GpSimd,
                        # levels 2-4 on DVE) and do a single 213ns broadcast
                        # ones-matmul on the reduced tile.
                        pe_den = qc == 3 and h >= 2
                        exp_sb = epool.tile([128, KT, 512], F32R, tag="exp")
                        av_ps = ps_av.tile([128, 512], F32, tag="av")
                        den_ps = ps_dn.tile([128, 512], F32, tag="den")
                        for kt in range(KT):
                            s_ps = ps_s.tile([128, 512], F32, tag="s")
                            nc.tensor.matmul(
                                s_ps[:],
                                kt_sb[:, kt * 128 : (kt + 1) * 128],
                                qt_all[:, h, qsl],
                                start=True,
                                stop=True,
                            )
                            nc.scalar.activation(exp_sb[:, kt, :], s_ps[:], AF.Exp)
                            if pe_den:
                                nc.tensor.matmul(
                                    den_ps[:],
                                    ones_sb[:],
                                    exp_sb[:, kt, :],
                                    start=(kt == 0),
                                    stop=(kt == KT - 1),
                                )
                            nc.tensor.matmul(
                                av_ps[:],
                                v_sb[:, kt, :],
                                exp_sb[:, kt, :],
                                start=(kt == 0),
                                stop=(kt == KT - 1),
                            )

                        if not pe_den:
                            w = 8
                            while w >= 1:
                                nc.vector.tensor_add(
                                    exp_sb[:, 0:w, :],
                                    exp_sb[:, 0:w, :],
                                    exp_sb[:, w : 2 * w, :],
                                )
                                w //= 2
                            nc.tensor.matmul(
                                den_ps[:],
                                ones_sb[:],
                                exp_sb[:, 0, :],
                                start=True,
                                stop=True,
                            )
                        rbc = dpool.tile([128, 512], F32, tag="rbc")
                        nc.vector.reciprocal_approx_fast(rbc[:], den_ps[:])
                        nc.vector.tensor_mul(outt_all[:, h, qsl], av_ps[:], rbc[:])

                def oproj(qc):
                    for st in range(4 * qc, 4 * qc + 4):
                        o_sb = opool.tile([128, 4, 512], BF, tag="o_sb")
                        for dc in range(4):
                            op_ps = ps_o.tile([128, 512], F32, tag="op")
                            for h in range(G):
                                nc.tensor.matmul(
                                    op_ps[:],
                                    outt_all[:, h, st * 128 : (st + 1) * 128],
                                    wo_sb[:, h, dc * 512 : (dc + 1) * 512],
                                    start=(h == 0),
                                    stop=(h == G - 1),
                                )
                            nc.vector.tensor_copy(o_sb[:, dc, :], op_ps[:])
                            nc.sync.dma_start(
                                bass.AP(
                                    tensor=o.tensor,
                                    offset=st * 128 * D + dc * 512,
                                    ap=[[D, 128], [1, 512]],
                                ),
                                o_sb[:, dc, :],
                            )

                attn(0)
                attn(1)
                oproj(0)
                attn(2)
                oproj(1)
                attn(3)
                oproj(2)
                oproj(3)

    nc.compile()
    return nc


def _rope_tables(qw, kw):
    """Folded cos/sin tables. RoPE rotation with rotate-half; per-head RMS
    norm weight w and the attention scale sc are folded in:
      out[d] = qhat[d]*w[d]*cos[d]*sc + qhat[(d+64)%128]*(sgn)*w[(d+64)%128]*sin[d]*sc
    where sgn = -1 for d < 64 (rotate-half negates the upper half moved down).
    """
    inv_freq = 1.0 / (ROPE_BASE ** (np.arange(0, HD, 2, dtype=np.float32) / HD))
    t = np.arange(S, dtype=np.float32)
    freqs = np.outer(t, inv_freq).astype(np.float32)  # [S, 64]
    emb = np.concatenate([freqs, freqs], axis=1)  # [S, 128]
    cos = np.cos(emb).astype(np.float32)
    sin = np.sin(emb).astype(np.float32)

    sgn = np.where(np.arange(HD) < 64, np.float32(-1.0), np.float32(1.0))
    wshift_q = np.roll(qw, -64)  # w[(d+64)%128]
    wshift_k = np.roll(kw, -64)
    sc = np.float32(1.0 / np.sqrt(HD))
    cq = cos * qw[None, :] * sc
    sq_ = sin * (sgn * wshift_q)[None, :] * sc
    ck = cos * kw[None, :]
    sk_ = sin * (sgn * wshift_k)[None, :]
    return cq, sq_, ck, sk_


def _sw_rows(m, nblk):
    """[nblk*128, L] row-major -> [128, nblk*L] partition-major (p = row % 128
    within block, line = concatenated blocks)."""
    L = m.shape[1]
    return np.ascontiguousarray(
        m.reshape(nblk, 128, L).transpose(1, 0, 2).reshape(128, nblk * L)
    )


def kernel(x, Wq, Wk, Wv, Wo, q_norm_w, k_norm_w):
    global _cached_nc, last_results
    import ml_dtypes

    bf16 = ml_dtypes.bfloat16
    x = np.asarray(x, dtype=np.float32)
    Wq = np.asarray(Wq, dtype=np.float32)
    Wk = np.asarray(Wk, dtype=np.float32)
    Wv = np.asarray(Wv, dtype=np.float32)
    Wo = np.asarray(Wo, dtype=np.float32)
    qw = np.asarray(q_norm_w, dtype=np.float32)
    kw = np.asarray(k_norm_w, dtype=np.float32)

    if _cached_nc is None:
        _cached_nc = _build_program()
    nc = _cached_nc

    cqt, sqt, ckt, skt = _rope_tables(qw, kw)

    in_maps = []
    for c in range(8):
        b, g = divmod(c, 4)
        # x.T pre-swizzled: xt_sw[p, st, ds, f] = x[b][st*128+f, ds*128+p]
        xt_sw = np.ascontiguousarray(
            x[b]
            .reshape(NT, 128, ND, 128)
            .transpose(3, 0, 2, 1)
            .reshape(128, NT * ND * 128)
            .astype(bf16)
        )
        wqkv_full = np.concatenate(
            [
                Wq[:, g * DQ : (g + 1) * DQ],
                Wk[:, g * HD : (g + 1) * HD],
                Wv[:, g * HD : (g + 1) * HD],
            ],
            axis=1,
        )
        in_maps.append(
            {
                "xt": xt_sw,
                "wqkv": _sw_rows(wqkv_full, ND).astype(bf16),
                "wo": _sw_rows(Wo[g * DQ : (g + 1) * DQ, :], G).astype(bf16),
                "cq": _sw_rows(cqt, NT).astype(bf16),
                "sq": _sw_rows(sqt, NT).astype(bf16),
                "ck": _sw_rows(ckt, NT).astype(bf16),
                "sk": _sw_rows(skt, NT).astype(bf16),
                "ones": np.ones((128, 128), dtype=np.float32),
            }
        )

    last_results = run_bass_kernel_spmd(nc, in_maps, core_ids=list(range(8)))

    out = np.zeros((B, S, D), dtype=np.float32)
    for c in range(8):
        b = c // 4
        out[b] += np.asarray(last_results.results[c]["o"], dtype=np.float32)
    return out
